# revision 12
# baseline (speedup 1.0000x reference)
"""Masked linear (CantorLinear): y = x @ (weight*mask).T + bias.

Structure exploited: the Cantor mask keeps ~3.9% of weights, arranged as 256
contiguous runs in the flattened (out, in) index space. Only 240 of the 2048
output rows have any nonzero weight. So the kernel packs those rows into a
[256, 2048] compact weight, computes the compact matmul on 8 NeuronCores
(data-parallel over the 16384 sequence positions), and scatters the 240
computed columns into a bias-broadcast full output on the host. The other
1808 output columns are exactly bias (filled host-side in fp32).

Device kernel (per core): y_cT[256, 2048] = W_cT.T @ x_T (+ bias per row),
with K = in_features on SBUF partitions for both operands; x is transposed
host-side so no on-device transpose is needed. Inputs stream as fp16
(x ~ N(0,1) and |W| <= 1/sqrt(2048) fit fp16's range; PSUM accumulates in
fp32), which halves HBM traffic vs fp32 at ~4.5e-4 absmax relative error.
Measured ~40 us/core steady state, near the max(PE 27us, HBM ~30us) floor.
"""

import os
import numpy as np

import concourse.bacc as bacc
import concourse.mybir as mybir
import concourse.tile as tile
from concourse.bass_utils import run_bass_kernel_spmd

B, SQ = 4, 4096
IN_F = 2048
OUT_F = 2048
S = B * SQ                 # 16384 flattened sequence positions
NCORES = 8
S_SH = S // NCORES         # 2048 per core
R_PAD = 256                # compact out-rows padded (240 real)
P = 128
KS = IN_F // P             # 16 k-subtiles
NT = int(os.environ.get("CANTOR_NT", "256"))   # sequence-tile width
MB = R_PAD // P            # 2 output partition blocks

# matmul input dtype: "fp16" (default), "bf16", "f32r", "f32"
MM_MODE = os.environ.get("CANTOR_MM_MODE", "fp16")
OUT_FP16 = os.environ.get("CANTOR_OUT_FP16", "1") == "1"
PRETILED = os.environ.get("CANTOR_PRETILED", "0") == "1"
# repeat the whole kernel body LOOPS times inside one NEFF (benchmarking only)
LOOPS = int(os.environ.get("CANTOR_BENCH_LOOPS", "1"))

LAST_RESULTS = None  # BassKernelResults of the most recent run (for test.py)

_NC_CACHE = {}


def _build_nc(mm_mode: str, loops: int):
    mm_cast = {
        "f32r": mybir.dt.float32r,
        "f32": mybir.dt.float32,
        "bf16": mybir.dt.bfloat16,
        "fp16": mybir.dt.float16,
    }[mm_mode]
    io_dt = mm_cast if mm_mode in ("bf16", "fp16") else mybir.dt.float32

    y_dt = mybir.dt.float16 if OUT_FP16 else mybir.dt.float32
    nc = bacc.Bacc("TRN2", target_bir_lowering=False, debug=False)
    n_si = S_SH // NT
    if PRETILED:
        xt = nc.dram_tensor("xt", [n_si, P, KS, NT], io_dt, kind="ExternalInput")
    else:
        xt = nc.dram_tensor("xt", [IN_F, S_SH], io_dt, kind="ExternalInput")
    wt = nc.dram_tensor("wt", [IN_F, R_PAD], io_dt, kind="ExternalInput")
    bc = nc.dram_tensor("bc", [R_PAD], mybir.dt.float32, kind="ExternalInput")
    yt = nc.dram_tensor("yt", [R_PAD, S_SH], y_dt, kind="ExternalOutput")

    if not PRETILED:
        xt_r = xt.rearrange("(ko p) s -> p ko s", p=P)
    wt_r = wt.rearrange("(ko p) r -> p ko r", p=P)
    bc_r = bc.rearrange("(m p) -> p m", p=P)

    with tile.TileContext(nc) as tc:
        is_f32r = mm_cast == mybir.dt.float32r
        with (
            tc.tile_pool(name="wpool", bufs=1) as wpool,
            tc.tile_pool(name="xpool", bufs=int(os.environ.get("CANTOR_XBUFS", "4"))) as xpool,
            tc.tile_pool(name="opool", bufs=int(os.environ.get("CANTOR_OBUFS", "4"))) as opool,
            tc.tile_pool(name="pspool", bufs=int(os.environ.get("CANTOR_PSBUFS", "4")), space="PSUM") as pspool,
        ):
            w_ld = wpool.tile([P, KS, R_PAD], io_dt)
            nc.sync.dma_start(w_ld[:], wt_r)
            b_sb = wpool.tile([P, MB], mybir.dt.float32)
            nc.sync.dma_start(b_sb[:], bc_r)
            if is_f32r:
                # fp32r matmul inputs must come from a rounding instruction.
                w_sb = wpool.tile([P, KS, R_PAD], mybir.dt.float32r)
                nc.vector.tensor_copy(w_sb[:], w_ld[:])
            else:
                w_sb = w_ld

            def body(_i=None):
                for si in range(S_SH // NT):
                    x_ld = xpool.tile([P, KS, NT], io_dt, tag="xld")
                    if PRETILED:
                        nc.sync.dma_start(x_ld[:], xt[si])
                    else:
                        nc.sync.dma_start(x_ld[:], xt_r[:, :, si * NT:(si + 1) * NT])
                    if is_f32r:
                        x_sb = xpool.tile([P, KS, NT], mybir.dt.float32r, tag="xr")
                        nc.vector.tensor_copy(x_sb[:], x_ld[:])
                    else:
                        x_sb = x_ld
                    for m in range(MB):
                        ps = pspool.tile([P, NT], mybir.dt.float32, tag="ps")
                        for k in range(KS):
                            nc.tensor.matmul(
                                ps[:],
                                lhsT=w_sb[:, k, m * P:(m + 1) * P],
                                rhs=x_sb[:, k, :],
                                start=(k == 0),
                                stop=(k == KS - 1),
                            )
                        o_sb = opool.tile([P, NT], y_dt, tag="o")
                        nc.scalar.activation(
                            o_sb[:], ps[:],
                            mybir.ActivationFunctionType.Identity,
                            bias=b_sb[:, m:m + 1],
                        )
                        nc.sync.dma_start(
                            yt[m * P:(m + 1) * P, si * NT:(si + 1) * NT], o_sb[:]
                        )

            if loops == 1:
                body()
            else:
                unroll = int(os.environ.get("CANTOR_BENCH_UNROLL", "1"))
                assert loops % unroll == 0
                hints = ()
                if os.environ.get("CANTOR_BENCH_HINTS", "0") == "1":
                    hints = (mybir.EngineType.PE, mybir.EngineType.SP)
                with tc.For_i(0, loops // unroll, 1, hint_engines=hints) as i:
                    for _ in range(unroll):
                        body(i)

    nc.compile()
    return nc


def _get_nc(mm_mode: str, loops: int):
    key = (mm_mode, loops)
    if key not in _NC_CACHE:
        _NC_CACHE[key] = _build_nc(mm_mode, loops)
    return _NC_CACHE[key]


def prep_in_maps(x, weight, bias, mask):
    """Host-side prep: pack compact weight/bias and per-core transposed x
    shards. Returns (in_maps, rows)."""
    x = np.asarray(x, dtype=np.float32)
    weight = np.asarray(weight, dtype=np.float32)
    bias = np.asarray(bias, dtype=np.float32)
    mask = np.asarray(mask, dtype=np.float32)

    w_eff = weight * mask
    rows = np.flatnonzero(mask.any(axis=1))
    r = len(rows)
    assert r <= R_PAD, f"compact rows {r} > padded {R_PAD}"

    if MM_MODE == "bf16":
        import ml_dtypes
        io_np = ml_dtypes.bfloat16
    elif MM_MODE == "fp16":
        io_np = np.float16
    else:
        io_np = np.float32

    w_c = np.zeros((R_PAD, IN_F), dtype=np.float32)
    w_c[:r] = w_eff[rows]
    wt = np.ascontiguousarray(w_c.T).astype(io_np)      # [IN_F, R_PAD]
    bc = np.zeros((R_PAD,), dtype=np.float32)
    bc[:r] = bias[rows]

    xf = x.reshape(S, IN_F)
    n_si = S_SH // NT
    in_maps = []
    for c in range(NCORES):
        x_t = xf[c * S_SH:(c + 1) * S_SH].T.astype(io_np)  # one-pass T + cast
        if PRETILED:
            # [IN_F, S_SH] -> [n_si, P, KS, NT]; partition-major contiguous
            x_t = np.ascontiguousarray(
                x_t.reshape(KS, P, n_si, NT).transpose(2, 1, 0, 3))
        in_maps.append({"xt": x_t, "wt": wt, "bc": bc})
    return in_maps, rows


def kernel(x, weight, bias, mask):
    global LAST_RESULTS
    bias = np.asarray(bias, dtype=np.float32)
    in_maps, rows = prep_in_maps(x, weight, bias, mask)
    r = len(rows)

    nc = _get_nc(MM_MODE, LOOPS)
    res = run_bass_kernel_spmd(nc, in_maps, list(range(NCORES)))
    LAST_RESULTS = res

    y = np.empty((S, OUT_F), dtype=np.float32)
    y[:] = bias
    for c in range(NCORES):
        y[c * S_SH:(c + 1) * S_SH, rows] = \
            res.results[c]["yt"][:r].T.astype(np.float32)
    return y.reshape(B, SQ, OUT_F)


# revision 14
# speedup vs baseline: 1.0412x; 1.0412x over previous
"""Masked linear (CantorLinear): y = x @ (weight*mask).T + bias.

Structure exploited: the Cantor mask keeps ~3.9% of weights, arranged as 256
contiguous runs in the flattened (out, in) index space. Only 240 of the 2048
output rows have any nonzero weight. So the kernel packs those rows into a
[256, 2048] compact weight, computes the compact matmul on 8 NeuronCores
(data-parallel over the 16384 sequence positions), and scatters the 240
computed columns into a bias-broadcast full output on the host. The other
1808 output columns are exactly bias (filled host-side in fp32).

Device kernel (per core): y_cT[256, 2048] = W_cT.T @ x_T (+ bias per row),
with K = in_features on SBUF partitions for both operands; x is transposed
host-side so no on-device transpose is needed. Inputs stream as fp16
(x ~ N(0,1) and |W| <= 1/sqrt(2048) fit fp16's range; PSUM accumulates in
fp32), which halves HBM traffic vs fp32 at ~4.5e-4 absmax relative error.
x is additionally pre-tiled host-side ([n_si, 128, 16, NT]) so every SBUF
tile DMA moves 8KB-contiguous per partition - HW-measured 8us faster than the
512B-chunk strided layout. Measured ~38.5 us/core steady state (PE-only floor
26.6us, DMA-only 25.8us; the rest is PE<->DMA latency coupling).
"""

import os
import numpy as np

import concourse.bacc as bacc
import concourse.mybir as mybir
import concourse.tile as tile
from concourse.bass_utils import run_bass_kernel_spmd

B, SQ = 4, 4096
IN_F = 2048
OUT_F = 2048
S = B * SQ                 # 16384 flattened sequence positions
NCORES = 8
S_SH = S // NCORES         # 2048 per core
R_PAD = 256                # compact out-rows padded (240 real)
P = 128
KS = IN_F // P             # 16 k-subtiles
NT = int(os.environ.get("CANTOR_NT", "512"))   # sequence-tile width
MB = R_PAD // P            # 2 output partition blocks

# matmul input dtype: "fp16" (default), "bf16", "f32r", "f32"
MM_MODE = os.environ.get("CANTOR_MM_MODE", "fp16")
OUT_FP16 = os.environ.get("CANTOR_OUT_FP16", "1") == "1"
PRETILED = os.environ.get("CANTOR_PRETILED", "1") == "1"
# repeat the whole kernel body LOOPS times inside one NEFF (benchmarking only)
LOOPS = int(os.environ.get("CANTOR_BENCH_LOOPS", "1"))

LAST_RESULTS = None  # BassKernelResults of the most recent run (for test.py)

_NC_CACHE = {}


def _build_nc(mm_mode: str, loops: int):
    mm_cast = {
        "f32r": mybir.dt.float32r,
        "f32": mybir.dt.float32,
        "bf16": mybir.dt.bfloat16,
        "fp16": mybir.dt.float16,
    }[mm_mode]
    io_dt = mm_cast if mm_mode in ("bf16", "fp16") else mybir.dt.float32

    y_dt = mybir.dt.float16 if OUT_FP16 else mybir.dt.float32
    nc = bacc.Bacc("TRN2", target_bir_lowering=False, debug=False)
    n_si = S_SH // NT
    if PRETILED:
        xt = nc.dram_tensor("xt", [n_si, P, KS, NT], io_dt, kind="ExternalInput")
    else:
        xt = nc.dram_tensor("xt", [IN_F, S_SH], io_dt, kind="ExternalInput")
    wt = nc.dram_tensor("wt", [IN_F, R_PAD], io_dt, kind="ExternalInput")
    bc = nc.dram_tensor("bc", [R_PAD], mybir.dt.float32, kind="ExternalInput")
    yt = nc.dram_tensor("yt", [R_PAD, S_SH], y_dt, kind="ExternalOutput")

    if not PRETILED:
        xt_r = xt.rearrange("(ko p) s -> p ko s", p=P)
    wt_r = wt.rearrange("(ko p) r -> p ko r", p=P)
    bc_r = bc.rearrange("(m p) -> p m", p=P)

    with tile.TileContext(nc) as tc:
        is_f32r = mm_cast == mybir.dt.float32r
        with (
            tc.tile_pool(name="wpool", bufs=1) as wpool,
            tc.tile_pool(name="xpool", bufs=int(os.environ.get("CANTOR_XBUFS", "4"))) as xpool,
            tc.tile_pool(name="opool", bufs=int(os.environ.get("CANTOR_OBUFS", "4"))) as opool,
            tc.tile_pool(name="pspool", bufs=int(os.environ.get("CANTOR_PSBUFS", "4")), space="PSUM") as pspool,
        ):
            w_ld = wpool.tile([P, KS, R_PAD], io_dt)
            nc.sync.dma_start(w_ld[:], wt_r)
            b_sb = wpool.tile([P, MB], mybir.dt.float32)
            nc.sync.dma_start(b_sb[:], bc_r)
            if is_f32r:
                # fp32r matmul inputs must come from a rounding instruction.
                w_sb = wpool.tile([P, KS, R_PAD], mybir.dt.float32r)
                nc.vector.tensor_copy(w_sb[:], w_ld[:])
            else:
                w_sb = w_ld

            ablate = os.environ.get("CANTOR_ABLATE", "")
            evict = os.environ.get("CANTOR_EVICT", "act")

            def body(_i=None):
                for si in range(S_SH // NT):
                    x_ld = xpool.tile([P, KS, NT], io_dt, tag="xld")
                    if ablate != "mm":
                        if PRETILED:
                            nc.sync.dma_start(x_ld[:], xt[si])
                        else:
                            nc.sync.dma_start(
                                x_ld[:], xt_r[:, :, si * NT:(si + 1) * NT])
                    else:
                        nc.any.memset(x_ld[:], 0.0)
                    if is_f32r:
                        x_sb = xpool.tile([P, KS, NT], mybir.dt.float32r, tag="xr")
                        nc.vector.tensor_copy(x_sb[:], x_ld[:])
                    else:
                        x_sb = x_ld
                    for m in range(MB):
                        o_sb = opool.tile([P, NT], y_dt, tag="o")
                        if ablate == "dma":
                            nc.any.memset(o_sb[:], 0.0)
                        else:
                            ps = pspool.tile([P, NT], mybir.dt.float32, tag="ps")
                            for k in range(KS):
                                nc.tensor.matmul(
                                    ps[:],
                                    lhsT=w_sb[:, k, m * P:(m + 1) * P],
                                    rhs=x_sb[:, k, :],
                                    start=(k == 0),
                                    stop=(k == KS - 1),
                                )
                            if evict == "dve":
                                nc.vector.tensor_tensor(
                                    o_sb[:], ps[:],
                                    b_sb[:, m:m + 1].to_broadcast([P, NT]),
                                    mybir.AluOpType.add,
                                )
                            else:
                                nc.scalar.activation(
                                    o_sb[:], ps[:],
                                    mybir.ActivationFunctionType.Identity,
                                    bias=b_sb[:, m:m + 1],
                                )
                        nc.sync.dma_start(
                            yt[m * P:(m + 1) * P, si * NT:(si + 1) * NT], o_sb[:]
                        )

            if loops == 1:
                body()
            else:
                unroll = int(os.environ.get("CANTOR_BENCH_UNROLL", "1"))
                assert loops % unroll == 0
                hints = ()
                if os.environ.get("CANTOR_BENCH_HINTS", "0") == "1":
                    hints = (mybir.EngineType.PE, mybir.EngineType.SP)
                with tc.For_i(0, loops // unroll, 1, hint_engines=hints) as i:
                    for _ in range(unroll):
                        body(i)

    nc.compile()
    return nc


def _get_nc(mm_mode: str, loops: int):
    key = (mm_mode, loops)
    if key not in _NC_CACHE:
        _NC_CACHE[key] = _build_nc(mm_mode, loops)
    return _NC_CACHE[key]


def prep_in_maps(x, weight, bias, mask):
    """Host-side prep: pack compact weight/bias and per-core transposed x
    shards. Returns (in_maps, rows)."""
    x = np.asarray(x, dtype=np.float32)
    weight = np.asarray(weight, dtype=np.float32)
    bias = np.asarray(bias, dtype=np.float32)
    mask = np.asarray(mask, dtype=np.float32)

    w_eff = weight * mask
    rows = np.flatnonzero(mask.any(axis=1))
    r = len(rows)
    assert r <= R_PAD, f"compact rows {r} > padded {R_PAD}"

    if MM_MODE == "bf16":
        import ml_dtypes
        io_np = ml_dtypes.bfloat16
    elif MM_MODE == "fp16":
        io_np = np.float16
    else:
        io_np = np.float32

    w_c = np.zeros((R_PAD, IN_F), dtype=np.float32)
    w_c[:r] = w_eff[rows]
    wt = np.ascontiguousarray(w_c.T).astype(io_np)      # [IN_F, R_PAD]
    bc = np.zeros((R_PAD,), dtype=np.float32)
    bc[:r] = bias[rows]

    xf = x.reshape(S, IN_F)
    n_si = S_SH // NT
    in_maps = []
    for c in range(NCORES):
        x_t = xf[c * S_SH:(c + 1) * S_SH].T.astype(io_np)  # one-pass T + cast
        if PRETILED:
            # [IN_F, S_SH] -> [n_si, P, KS, NT]; partition-major contiguous
            x_t = np.ascontiguousarray(
                x_t.reshape(KS, P, n_si, NT).transpose(2, 1, 0, 3))
        in_maps.append({"xt": x_t, "wt": wt, "bc": bc})
    return in_maps, rows


def kernel(x, weight, bias, mask):
    global LAST_RESULTS
    bias = np.asarray(bias, dtype=np.float32)
    in_maps, rows = prep_in_maps(x, weight, bias, mask)
    r = len(rows)

    nc = _get_nc(MM_MODE, LOOPS)
    res = run_bass_kernel_spmd(nc, in_maps, list(range(NCORES)))
    LAST_RESULTS = res

    y = np.empty((S, OUT_F), dtype=np.float32)
    y[:] = bias
    for c in range(NCORES):
        y[c * S_SH:(c + 1) * S_SH, rows] = \
            res.results[c]["yt"][:r].T.astype(np.float32)
    return y.reshape(B, SQ, OUT_F)
